# revision 40
# baseline (speedup 1.0000x reference)
"""NTN kernel: f16 stream with the bias folded into the input encoding.

y = relu(x1 @ M + c) @ u,  M = V[:,:D] + (W @ x2)^T  (128x16),
c = x2 @ V[:,D:]^T + b,    u = U[:,0].

Three host-side folds make the device program trivial:

1. u into M:   u_k relu(z_k + c_k) = s_k relu(|u_k| z_k + |u_k| c_k),
   so M2 = M diag|u| with positive-u columns permuted first (KP of
   them) and y = sum(first KP) - sum(rest).
2. c into x:   pick the min-norm row offset D with D @ M2 = c2
   (pinv of the well-conditioned random 128x16 M2; ||D|| ~ 0.3).
   Streaming x~ = x1 + D makes the matmul produce z + c directly --
   no bias matmul, no threshold tables on device.
3. f16 encode: x~ and M2 stream as f16 (2 B/elem -> 16 MB/core,
   rel err ~5e-4 vs tolerance 2e-2).

Device, per 64-tile group (one DMA chunk):
    PE:   one 128x128 @ 128x16 f16 matmul per row-tile into PSUM
    ACT:  relu straight out of PSUM into an f16 elem tile (also
          releases the PSUM bank; ACT is otherwise idle)
    DVE:  plain-sum reduce over first KP columns -> y, reduce over
          the rest -> rn, y -= rn   (f16 input = 2x DVE throughput)
    Pool: single y store at the very end
    SP:   mt param DMA first (any other queue's tiny descriptors
          starve ~10us behind the stream), then all x chunks on the
          single sync queue.

Engine loads per 4.9us group cadence: PE 1.7us, ACT 1.2, DVE 1.0 --
nothing rate-limits but the DMA stream itself.  Hard-won DMA facts
baked into this shape: one HWDGE queue saturates all 16 DMA engines
(~420 B/ns; a second concurrent queue slows BOTH by ~33%); the 16
engine FIFOs are shared across queues, descriptors enqueue at issue
time, so mid-stream y stores either delay the stream or gain nothing;
per run, one engine tends to be ~20% slow, which is why chunk
completions trail their last data packet.  The last groups use an
all-DVE epilogue (relu in place on PSUM) to drop the ACT hop from the
post-stream critical path, and the final chunks are small so little
work trails the last byte.  KP depends on the sign pattern of u, so
the program is built after inputs arrive (cached per KP).
"""

import numpy as np

import concourse.bass as bass
import concourse.bacc as bacc
import concourse.mybir as mybir
import concourse.tile as tile

N, D, K = 500000, 128, 16
NCORES = 8
ROWS_PER_CORE = N // NCORES
TILES = 489
RPC = TILES * 128
GROUP = 64
DMA_CHUNK = 64
F32 = mybir.dt.float32
F16 = mybir.dt.float16


def _build_program(kp):
    nc = bacc.Bacc(None, target_bir_lowering=False)

    # Columns 0..K-1 of xh hold M2 itself; the x tiles start at col K.
    # Folding the 4KB parameter into chunk 0 removes a separate
    # dma_start (its 128 tiny descriptors, 16 completion sems, and
    # 0.65us of issue time ahead of the stream).
    xh = nc.dram_tensor("xh", [128, K + RPC], F16, kind="ExternalInput")
    y = nc.dram_tensor("y", [128, TILES], F32, kind="ExternalOutput")

    with tile.TileContext(nc) as tc:
        with (
            tc.tile_pool(name="singles", bufs=1) as singles,
            tc.tile_pool(name="xin", bufs=3) as xin,
            tc.tile_pool(name="zp", bufs=3, space="PSUM") as zpool,
            tc.tile_pool(name="work", bufs=4) as work,
            tc.tile_pool(name="yout", bufs=1) as yout,
        ):
            # Chunk 0 (64 tiles + the K cols of M2) is pinned for the
            # whole run; mid chunks are big (fewer dma_starts -> fewer
            # completion sems to drain at teardown), tail chunks small.
            sizes = [64, 128, 128, 128, 16, 16, 9]
            assert sum(sizes) == TILES

            # Single sync HWDGE queue: it saturates the 16 DMA engines,
            # and a second concurrent queue was measured to slow BOTH
            # queues' descriptors by ~33%.
            c0_t = singles.tile([128, K + 64 * 128], F16)
            nc.sync.dma_start(c0_t, xh[:, : K + 64 * 128])
            mt_sb = c0_t[:, :K]
            chunk_tiles = [(0, 64, c0_t, K)]
            c0 = 64
            for nct in sizes[1:]:
                xh_t = xin.tile([128, 128 * 128], F16, tag="xh")
                nc.sync.dma_start(
                    xh_t[:, : nct * 128],
                    xh[:, K + c0 * 128 : K + (c0 + nct) * 128],
                )
                chunk_tiles.append((c0, nct, xh_t, 0))
                c0 += nct
            assert c0 == TILES

            # y accumulates in SBUF and is stored ONCE at the end.  Any
            # mid-stream y store costs far more than it saves: its 128
            # small descriptors sit in a few DMA engines' FIFOs and
            # delay that chunk's completion-semaphore increments by
            # 1.5-4us, stalling the matmuls behind a phantom "slow DMA".
            y_sb = yout.tile([128, TILES], F32)

            TAIL0 = 448           # groups past here use the DVE epilogue

            for c0, nct, xh_t, off in chunk_tiles:
                g0 = 0
                while g0 < nct:
                    nt = min(GROUP, nct - g0)
                    t0 = c0 + g0
                    tail = t0 >= TAIL0
                    zp = zpool.tile([128, GROUP, K], F32, tag="z")
                    for t in range(nt):
                        sl = slice(
                            off + (g0 + t) * 128, off + (g0 + t + 1) * 128
                        )
                        nc.tensor.matmul(
                            zp[:, t, :], xh_t[:, sl], mt_sb[:, :],
                            start=True, stop=True,
                        )
                    if tail:
                        # Tail epilogue skips the ACT hop: relu in place
                        # on PSUM, reduce from PSUM.  One less engine on
                        # the post-stream critical path.
                        nc.vector.tensor_scalar_max(
                            zp[:, :nt, :], zp[:, :nt, :], 0.0
                        )
                        src = zp
                    else:
                        elem = work.tile([128, GROUP, K], F16, tag="elem")
                        nc.scalar.activation(
                            elem[:, :nt, :], zp[:, :nt, :],
                            mybir.ActivationFunctionType.Relu,
                        )
                        src = elem
                    ysl = y_sb[:, t0 : t0 + nt]
                    if kp > 0:
                        nc.vector.tensor_reduce(
                            ysl, src[:, :nt, :kp],
                            axis=mybir.AxisListType.X, op=mybir.AluOpType.add,
                        )
                    if kp < K:
                        rn = work.tile([128, GROUP], F32, tag="rn")
                        nc.vector.tensor_reduce(
                            rn[:, :nt], src[:, :nt, kp:],
                            axis=mybir.AxisListType.X, op=mybir.AluOpType.add,
                        )
                        if kp > 0:
                            nc.vector.tensor_tensor(
                                ysl, ysl, rn[:, :nt],
                                op=mybir.AluOpType.subtract,
                            )
                        else:
                            nc.vector.tensor_scalar_mul(
                                ysl, rn[:, :nt], -1.0
                            )
                    g0 += nt

            # y is stored only at the very end.  The 16 DMA-engine FIFOs
            # are shared across ALL queues, so a mid-stream store either
            # delays the x stream (issued early -> descriptors jump in
            # front of later x chunks) or waits behind the entire x
            # backlog anyway (issued late).  At the end the FIFOs are
            # empty, so split the store across two queues in parallel.
            nc.sync.dma_start(y[:, : TILES // 2], y_sb[:, : TILES // 2])
            nc.gpsimd.dma_start(y[:, TILES // 2 :], y_sb[:, TILES // 2 :])

    nc.compile()
    return nc


_NC_CACHE = {}


def _get_program(kp):
    if kp not in _NC_CACHE:
        _NC_CACHE[kp] = _build_program(kp)
    return _NC_CACHE[kp]


def _host_prep(x1, x2, V, W, b, U):
    x1 = np.asarray(x1, dtype=np.float32)
    x2 = np.asarray(x2, dtype=np.float64)
    V = np.asarray(V, dtype=np.float64)
    W = np.asarray(W, dtype=np.float64)
    b = np.asarray(b, dtype=np.float64)
    U = np.asarray(U, dtype=np.float64)

    M = V[:, :D] + np.einsum("kde,e->kd", W, x2[0])   # (K, D)
    c = (x2[0] @ V[:, D:].T) + b                      # (K,)
    u = U[:, 0]                                       # (K,)

    perm = np.argsort(u <= 0, kind="stable")
    kp = int(np.sum(u > 0))
    up = np.abs(u[perm])
    M2 = (M[perm] * up[:, None]).T                    # (D, K)
    c2 = c[perm] * up                                 # (K,)
    delta = np.linalg.pinv(M2.T) @ c2                 # (D,) min-norm offset

    mt = M2.astype(np.float16)

    in_maps = []
    for cidx in range(NCORES):
        sl = x1[cidx * ROWS_PER_CORE : (cidx + 1) * ROWS_PER_CORE]
        hbuf = np.zeros((128, K + RPC), dtype=np.float16)
        hbuf[:, :K] = mt
        hbuf[:, K : K + ROWS_PER_CORE] = (
            sl.T + delta[:, None].astype(np.float32)
        ).astype(np.float16)
        in_maps.append({"xh": hbuf})
    return in_maps, kp


def _gather(results):
    outs = []
    for cidx in range(NCORES):
        yc = np.asarray(results[cidx]["y"])
        outs.append(yc.T.reshape(-1)[:ROWS_PER_CORE])
    return np.concatenate(outs).reshape(N, 1).astype(np.float32)


def run_device(in_maps, kp, trace=False):
    from concourse.bass_utils import run_bass_kernel_spmd

    nc = _get_program(kp)
    res = run_bass_kernel_spmd(
        nc, in_maps, core_ids=list(range(NCORES)), trace=trace
    )
    return res


def kernel(x1, x2, V, W, b, U):
    in_maps, kp = _host_prep(x1, x2, V, W, b, U)
    res = run_device(in_maps, kp, trace=False)
    return _gather(res.results)


# revision 41
# speedup vs baseline: 1.0032x; 1.0032x over previous
"""NTN kernel: f16 stream with the bias folded into the input encoding.

y = relu(x1 @ M + c) @ u,  M = V[:,:D] + (W @ x2)^T  (128x16),
c = x2 @ V[:,D:]^T + b,    u = U[:,0].

Three host-side folds make the device program trivial:

1. u into M:   u_k relu(z_k + c_k) = s_k relu(|u_k| z_k + |u_k| c_k),
   so M2 = M diag|u| with positive-u columns permuted first (KP of
   them) and y = sum(first KP) - sum(rest).
2. c into x:   pick the min-norm row offset D with D @ M2 = c2
   (pinv of the well-conditioned random 128x16 M2; ||D|| ~ 0.3).
   Streaming x~ = x1 + D makes the matmul produce z + c directly --
   no bias matmul, no threshold tables on device.
3. f16 encode: x~ and M2 stream as f16 (2 B/elem -> 16 MB/core,
   rel err ~5e-4 vs tolerance 2e-2).

Device, per 64-tile group (one DMA chunk):
    PE:   one 128x128 @ 128x16 f16 matmul per row-tile into PSUM
    ACT:  relu straight out of PSUM into an f16 elem tile (also
          releases the PSUM bank; ACT is otherwise idle)
    DVE:  plain-sum reduce over first KP columns -> y, reduce over
          the rest -> rn, y -= rn   (f16 input = 2x DVE throughput)
    Pool: single y store at the very end
    SP:   mt param DMA first (any other queue's tiny descriptors
          starve ~10us behind the stream), then all x chunks on the
          single sync queue.

Engine loads per 4.9us group cadence: PE 1.7us, ACT 1.2, DVE 1.0 --
nothing rate-limits but the DMA stream itself.  Hard-won DMA facts
baked into this shape: one HWDGE queue saturates all 16 DMA engines
(~420 B/ns; a second concurrent queue slows BOTH by ~33%); the 16
engine FIFOs are shared across queues, descriptors enqueue at issue
time, so mid-stream y stores either delay the stream or gain nothing;
per run, one engine tends to be ~20% slow, which is why chunk
completions trail their last data packet.  The last groups use an
all-DVE epilogue (relu in place on PSUM) to drop the ACT hop from the
post-stream critical path, and the final chunks are small so little
work trails the last byte.  KP depends on the sign pattern of u, so
the program is built after inputs arrive (cached per KP).
"""

import numpy as np

import concourse.bass as bass
import concourse.bacc as bacc
import concourse.mybir as mybir
import concourse.tile as tile

N, D, K = 500000, 128, 16
NCORES = 8
ROWS_PER_CORE = N // NCORES
TILES = 489
RPC = TILES * 128
GROUP = 64
DMA_CHUNK = 64
F32 = mybir.dt.float32
F16 = mybir.dt.float16


def _build_program(kp):
    nc = bacc.Bacc(None, target_bir_lowering=False)

    # Columns 0..K-1 of xh hold M2 itself; the x tiles start at col K.
    # Folding the 4KB parameter into chunk 0 removes a separate
    # dma_start (its 128 tiny descriptors, 16 completion sems, and
    # 0.65us of issue time ahead of the stream).
    xh = nc.dram_tensor("xh", [128, K + RPC], F16, kind="ExternalInput")
    y = nc.dram_tensor("y", [128, TILES], F32, kind="ExternalOutput")

    with tile.TileContext(nc) as tc:
        with (
            tc.tile_pool(name="singles", bufs=1) as singles,
            tc.tile_pool(name="xin", bufs=3) as xin,
            tc.tile_pool(name="zp", bufs=3, space="PSUM") as zpool,
            tc.tile_pool(name="work", bufs=4) as work,
            tc.tile_pool(name="yout", bufs=1) as yout,
        ):
            # Chunk 0 (64 tiles + the K cols of M2) is pinned for the
            # whole run; mid chunks are big (fewer dma_starts -> fewer
            # completion sems to drain at teardown), tail chunks small.
            sizes = [64, 128, 128, 128, 16, 16, 9]
            assert sum(sizes) == TILES

            # Single sync HWDGE queue: it saturates the 16 DMA engines,
            # and a second concurrent queue was measured to slow BOTH
            # queues' descriptors by ~33%.
            c0_t = singles.tile([128, K + 64 * 128], F16)
            nc.sync.dma_start(c0_t, xh[:, : K + 64 * 128])
            mt_sb = c0_t[:, :K]
            chunk_tiles = [(0, 64, c0_t, K)]
            c0 = 64
            for nct in sizes[1:]:
                xh_t = xin.tile([128, 128 * 128], F16, tag="xh")
                nc.sync.dma_start(
                    xh_t[:, : nct * 128],
                    xh[:, K + c0 * 128 : K + (c0 + nct) * 128],
                )
                chunk_tiles.append((c0, nct, xh_t, 0))
                c0 += nct
            assert c0 == TILES

            # y accumulates in SBUF and is stored ONCE at the end.  Any
            # mid-stream y store costs far more than it saves: its 128
            # small descriptors sit in a few DMA engines' FIFOs and
            # delay that chunk's completion-semaphore increments by
            # 1.5-4us, stalling the matmuls behind a phantom "slow DMA".
            y_sb = yout.tile([128, TILES], F32)

            TAIL0 = 448           # groups past here use the DVE epilogue

            for c0, nct, xh_t, off in chunk_tiles:
                g0 = 0
                while g0 < nct:
                    nt = min(GROUP, nct - g0)
                    t0 = c0 + g0
                    tail = t0 >= TAIL0
                    zp = zpool.tile([128, GROUP, K], F32, tag="z")
                    for t in range(nt):
                        sl = slice(
                            off + (g0 + t) * 128, off + (g0 + t + 1) * 128
                        )
                        nc.tensor.matmul(
                            zp[:, t, :], xh_t[:, sl], mt_sb[:, :],
                            start=True, stop=True,
                        )
                    if tail:
                        # Tail epilogue skips the ACT hop: relu in place
                        # on PSUM, reduce from PSUM.  One less engine on
                        # the post-stream critical path.
                        nc.vector.tensor_scalar_max(
                            zp[:, :nt, :], zp[:, :nt, :], 0.0
                        )
                        src = zp
                    else:
                        elem = work.tile([128, GROUP, K], F16, tag="elem")
                        nc.scalar.activation(
                            elem[:, :nt, :], zp[:, :nt, :],
                            mybir.ActivationFunctionType.Relu,
                        )
                        src = elem
                    ysl = y_sb[:, t0 : t0 + nt]
                    if kp > 0:
                        nc.vector.tensor_reduce(
                            ysl, src[:, :nt, :kp],
                            axis=mybir.AxisListType.X, op=mybir.AluOpType.add,
                        )
                    if kp < K:
                        rn = work.tile([128, GROUP], F32, tag="rn")
                        nc.vector.tensor_reduce(
                            rn[:, :nt], src[:, :nt, kp:],
                            axis=mybir.AxisListType.X, op=mybir.AluOpType.add,
                        )
                        if kp > 0:
                            nc.vector.tensor_tensor(
                                ysl, ysl, rn[:, :nt],
                                op=mybir.AluOpType.subtract,
                            )
                        else:
                            nc.vector.tensor_scalar_mul(
                                ysl, rn[:, :nt], -1.0
                            )
                    g0 += nt

            # y is stored only after the stream.  The 16 DMA-engine FIFOs
            # are shared across ALL queues, so an early store's
            # descriptors would jump in front of later x chunks and
            # stall them.  Emitted here (after every x issue), the big
            # [0:TAIL0] store fires once group TAIL0-1 is reduced and its
            # descriptors drain right behind the stream, overlapping the
            # tail compute; only the last 41 tiles ride the critical
            # path, on the empty Pool queue.
            nc.sync.dma_start(y[:, :TAIL0], y_sb[:, :TAIL0])
            nc.gpsimd.dma_start(y[:, TAIL0:], y_sb[:, TAIL0:])

    nc.compile()
    return nc


_NC_CACHE = {}


def _get_program(kp):
    if kp not in _NC_CACHE:
        _NC_CACHE[kp] = _build_program(kp)
    return _NC_CACHE[kp]


def _host_prep(x1, x2, V, W, b, U):
    x1 = np.asarray(x1, dtype=np.float32)
    x2 = np.asarray(x2, dtype=np.float64)
    V = np.asarray(V, dtype=np.float64)
    W = np.asarray(W, dtype=np.float64)
    b = np.asarray(b, dtype=np.float64)
    U = np.asarray(U, dtype=np.float64)

    M = V[:, :D] + np.einsum("kde,e->kd", W, x2[0])   # (K, D)
    c = (x2[0] @ V[:, D:].T) + b                      # (K,)
    u = U[:, 0]                                       # (K,)

    perm = np.argsort(u <= 0, kind="stable")
    kp = int(np.sum(u > 0))
    up = np.abs(u[perm])
    M2 = (M[perm] * up[:, None]).T                    # (D, K)
    c2 = c[perm] * up                                 # (K,)
    delta = np.linalg.pinv(M2.T) @ c2                 # (D,) min-norm offset

    mt = M2.astype(np.float16)

    in_maps = []
    for cidx in range(NCORES):
        sl = x1[cidx * ROWS_PER_CORE : (cidx + 1) * ROWS_PER_CORE]
        hbuf = np.zeros((128, K + RPC), dtype=np.float16)
        hbuf[:, :K] = mt
        hbuf[:, K : K + ROWS_PER_CORE] = (
            sl.T + delta[:, None].astype(np.float32)
        ).astype(np.float16)
        in_maps.append({"xh": hbuf})
    return in_maps, kp


def _gather(results):
    outs = []
    for cidx in range(NCORES):
        yc = np.asarray(results[cidx]["y"])
        outs.append(yc.T.reshape(-1)[:ROWS_PER_CORE])
    return np.concatenate(outs).reshape(N, 1).astype(np.float32)


def run_device(in_maps, kp, trace=False):
    from concourse.bass_utils import run_bass_kernel_spmd

    nc = _get_program(kp)
    res = run_bass_kernel_spmd(
        nc, in_maps, core_ids=list(range(NCORES)), trace=trace
    )
    return res


def kernel(x1, x2, V, W, b, U):
    in_maps, kp = _host_prep(x1, x2, V, W, b, U)
    res = run_device(in_maps, kp, trace=False)
    return _gather(res.results)
